# revision 3
# baseline (speedup 1.0000x reference)
"""Causal self-attention (B=4, T=2048, C=1024, NH=16) on 8 trn2 NeuronCores.

Sharding: core = (head_group hg in {0,1}) x (batch b in {0..3}).
Each core computes qkv projection + attention + partial output projection for
its 8 heads of its batch; host sums the two head-group partials per batch and
adds the output bias.

Layout strategy (all matmuls in float32r, ~TF32 accuracy, 4x fp32 speed):
  - qkv computed transposed: qkvT = W_slice @ x[b].T, so attention operands
    have head_size on partitions (contraction dim of QK^T).
  - S^T = K @ Q^T computed per (head, 128-key-tile, 256-query-tile), exp on
    ScalarE (no max subtraction; scores are O(5) so exp is safe), causal mask
    applied as a 0/1 multiply on the two diagonal blocks only.
  - V is re-transposed to natural [t, hs] layout via PE transposes and
    augmented with a ones column, so the AV matmul also produces the softmax
    denominator for free: [V | 1]^T @ P^T = [O^T ; d].
  - 1/d is broadcast across partitions with a K=1 matmul against a ones
    vector, then applied with a vector multiply.
  - output projection contracts the core's 512 o-features; partial [T, C]
    result is summed on the host.
"""

import sys

sys.path.insert(0, "/opt/trn_rl_repo")

import numpy as np

import concourse.bacc as bacc
import concourse.bass as bass
import concourse.mybir as mybir
from concourse.bass_utils import run_bass_kernel_spmd
from concourse.masks import make_identity
from concourse.tile import TileContext

B, T, C, NH = 4, 2048, 1024, 16
HS = C // NH          # 64
HGF = 512             # features per head group (8 heads x 64)
QT = 256              # query tile (attention)
NJ = T // QT          # 8 query tiles
NKT = T // 128        # 16 key tiles
F32 = mybir.dt.float32
F32R = mybir.dt.float32r


def build_kernel():
    nc = bacc.Bacc(None, target_bir_lowering=False)
    xT = nc.dram_tensor("xT", (C, T), F32R, kind="ExternalInput")
    wqkvT = nc.dram_tensor("wqkvT", (C, 3 * HGF), F32R, kind="ExternalInput")
    bqkv = nc.dram_tensor("bqkv", (12, 128, 1), F32, kind="ExternalInput")
    wprojT = nc.dram_tensor("wprojT", (HGF, C), F32R, kind="ExternalInput")
    mask01 = nc.dram_tensor("mask01", (QT, QT), F32R, kind="ExternalInput")
    y = nc.dram_tensor("y", (T, C), F32, kind="ExternalOutput")

    with TileContext(nc) as tc:
        with (
            tc.tile_pool(name="outer", bufs=1) as outer,
            tc.tile_pool(name="psum", bufs=1, space="PSUM") as psum,
        ):
            ident = outer.tile([128, 128], F32)
            make_identity(nc, ident)
            ones_f = outer.tile([128, 2], F32)
            nc.vector.memset(ones_f, 1.0)
            ones2 = outer.tile([128, 2], F32R)
            nc.vector.tensor_copy(ones2, ones_f)
            onesk_f = outer.tile([1, 64], F32)
            nc.vector.memset(onesk_f, 1.0)
            ones_k1 = outer.tile([1, 64], F32R)
            nc.vector.tensor_copy(ones_k1, onesk_f)

            # resident across both phases
            q_t = [outer.tile([128, T], F32R, name=f"q{i}") for i in range(4)]
            k_t = [outer.tile([128, T], F32R, name=f"k{i}") for i in range(4)]
            # v_nat[pair][tk]: [tk 128, 130] = [headA 64 | 1 | headB 64 | 1]
            vnat = [
                [outer.tile([128, 130], F32R, name=f"v{p}_{i}") for i in range(NKT)]
                for p in range(4)
            ]

            # ---------------- phase 1: qkv projection (+ V transpose) -------
            with tc.tile_pool(name="ph1", bufs=1) as ph1:
                w_t = [ph1.tile([128, 3 * HGF], F32R, name=f"w{k}") for k in range(8)]
                for k in range(8):
                    nc.sync.dma_start(w_t[k], wqkvT[k * 128:(k + 1) * 128, :])
                bias_t = []
                for m in range(12):
                    bt = ph1.tile([128, 1], F32, name=f"b{m}")
                    nc.sync.dma_start(bt, bqkv[m])
                    bias_t.append(bt)

                for n in range(4):  # 512-wide t tiles
                    x_n = []
                    for k in range(8):
                        # one tag per k: all 8 k-tiles of an n-tile are live at
                        # once, so they must not compete for one tag's slots
                        xt = ph1.tile([128, 512], F32R, tag=f"x{k}", bufs=2,
                                      name=f"x{n}_{k}")
                        nc.sync.dma_start(
                            xt, xT[k * 128:(k + 1) * 128, n * 512:(n + 1) * 512]
                        )
                        x_n.append(xt)
                    for m in range(12):
                        ps = psum.tile([128, 512], F32, tag="mm512", bufs=2)
                        for k in range(8):
                            nc.tensor.matmul(
                                ps,
                                w_t[k][:, m * 128:(m + 1) * 128],
                                x_n[k],
                                start=(k == 0),
                                stop=(k == 7),
                            )
                        if m < 8:
                            dst = q_t[m] if m < 4 else k_t[m - 4]
                            nc.vector.tensor_scalar_add(
                                dst[:, n * 512:(n + 1) * 512], ps, bias_t[m]
                            )
                        else:
                            pair = m - 8
                            vtmp = ph1.tile([128, 512], F32, tag="vtmp", bufs=2)
                            nc.vector.tensor_scalar_add(vtmp, ps, bias_t[m])
                            for t4 in range(4):
                                tk = 4 * n + t4
                                pst = psum.tile([128, 128], F32, tag="small", bufs=4)
                                nc.tensor.transpose(
                                    pst, vtmp[:, t4 * 128:(t4 + 1) * 128], ident
                                )
                                vt = vnat[pair][tk]
                                # data cols {0..63, 65..128}
                                nc.vector.tensor_copy(
                                    vt[:, 0:130].rearrange("p (g c) -> p g c", c=65)[:, :, 0:64],
                                    pst.rearrange("p (g c) -> p g c", c=64),
                                )
                                nc.vector.tensor_copy(
                                    vt[:, 0:130].rearrange("p (g c) -> p g c", c=65)[:, :, 64:65],
                                    ones2.rearrange("p (g c) -> p g c", c=1),
                                )

            # ---------------- phase 2: attention + projection ---------------
            with tc.tile_pool(name="ph2", bufs=1) as ph2:
                wp_t = [ph2.tile([128, C], F32R, name=f"wp{k}") for k in range(4)]
                for k in range(4):
                    nc.sync.dma_start(wp_t[k], wprojT[k * 128:(k + 1) * 128, :])
                mask_t = []
                for i in range(2):
                    mt = ph2.tile([128, QT], F32R, name=f"mask{i}")
                    nc.sync.dma_start(mt, mask01[i * 128:(i + 1) * 128, :])
                    mask_t.append(mt)

                for j in range(NJ):
                    o_j = [
                        ph2.tile([128, QT], F32R, tag=f"o{i}", bufs=2, name=f"o{i}_{j}")
                        for i in range(4)
                    ]
                    for h in range(8):
                        pair, off = h // 2, 64 * (h % 2)
                        n_tk = 2 * (j + 1)
                        po = psum.tile([65, QT], F32, tag="o", bufs=2)
                        for i in range(n_tk):
                            ps_s = psum.tile([128, QT], F32, tag="small", bufs=4)
                            nc.tensor.matmul(
                                ps_s,
                                k_t[pair][off:off + 64, i * 128:(i + 1) * 128],
                                q_t[pair][off:off + 64, j * QT:(j + 1) * QT],
                                start=True,
                                stop=True,
                            )
                            pt = ph2.tile([128, QT], F32R, tag="pt", bufs=4)
                            nc.scalar.activation(
                                pt, ps_s, mybir.ActivationFunctionType.Exp, scale=0.125
                            )
                            if i >= 2 * j:
                                nc.vector.tensor_mul(pt, pt, mask_t[i - 2 * j])
                            nc.tensor.matmul(
                                po,
                                vnat[pair][i][:, 65 * (h % 2):65 * (h % 2) + 65],
                                pt,
                                start=(i == 0),
                                stop=(i == n_tk - 1),
                            )
                        dinv_f = ph2.tile([1, QT], F32, tag="dinvf", bufs=2)
                        nc.vector.reciprocal(dinv_f, po[64:65, :])
                        dinv_r = ph2.tile([1, QT], F32R, tag="dinvr", bufs=2)
                        nc.vector.tensor_copy(dinv_r, dinv_f)
                        pb = psum.tile([64, QT], F32, tag="small", bufs=4)
                        nc.tensor.matmul(pb, ones_k1, dinv_r, start=True, stop=True)
                        bc = ph2.tile([64, QT], F32, tag="bc", bufs=2)
                        nc.vector.tensor_copy(bc, pb)
                        nc.vector.tensor_mul(o_j[pair][off:off + 64, :], po[0:64, :], bc)
                    for mm in range(2):
                        for nn in range(2):
                            psy = psum.tile([128, 512], F32, tag="mm512", bufs=2)
                            for k4 in range(4):
                                nc.tensor.matmul(
                                    psy,
                                    o_j[k4][:, mm * 128:(mm + 1) * 128],
                                    wp_t[k4][:, nn * 512:(nn + 1) * 512],
                                    start=(k4 == 0),
                                    stop=(k4 == 3),
                                )
                            ysb = ph2.tile([128, 512], F32, tag="ysb", bufs=3)
                            nc.vector.tensor_copy(ysb, psy)
                            nc.sync.dma_start(
                                y[
                                    j * QT + mm * 128:j * QT + (mm + 1) * 128,
                                    nn * 512:(nn + 1) * 512,
                                ],
                                ysb,
                            )

    nc.finalize()
    return nc


_NC = None


def _get_nc():
    global _NC
    if _NC is None:
        _NC = build_kernel()
    return _NC


def kernel(x, Wqkv, bqkv, Wproj, bproj, _trace=False):
    x = np.asarray(x, dtype=np.float32)
    Wqkv = np.asarray(Wqkv, dtype=np.float32)
    bqkv = np.asarray(bqkv, dtype=np.float32)
    Wproj = np.asarray(Wproj, dtype=np.float32)
    bproj = np.asarray(bproj, dtype=np.float32)

    mask = np.triu(np.ones((QT, QT), dtype=np.float32))  # allow key <= query
    in_maps = []
    for hg in range(2):
        sl = slice(hg * HGF, (hg + 1) * HGF)
        rows = np.concatenate([Wqkv[sl], Wqkv[1024 + hg * HGF:1024 + (hg + 1) * HGF],
                               Wqkv[2048 + hg * HGF:2048 + (hg + 1) * HGF]])
        wqkvT = np.ascontiguousarray(rows.T)  # [C, 1536]
        bq = np.concatenate(
            [bqkv[sl], bqkv[1024 + hg * HGF:1024 + (hg + 1) * HGF],
             bqkv[2048 + hg * HGF:2048 + (hg + 1) * HGF]]
        ).reshape(12, 128, 1)
        wprojT = np.ascontiguousarray(Wproj[:, sl].T)  # [512, C]
        for b in range(B):
            in_maps.append(
                {
                    "xT": np.ascontiguousarray(x[b].T),
                    "wqkvT": wqkvT,
                    "bqkv": bq,
                    "wprojT": wprojT,
                    "mask01": mask,
                }
            )
    # core order: idx = hg * 4 + b
    res = run_bass_kernel_spmd(_get_nc(), in_maps, core_ids=list(range(8)),
                               trace=_trace)
    out = np.empty((B, T, C), dtype=np.float32)
    for b in range(B):
        out[b] = res.results[b]["y"] + res.results[4 + b]["y"] + bproj
    if _trace:
        return out, res
    return out


# revision 14
# speedup vs baseline: 1.0006x; 1.0006x over previous
"""Causal self-attention (B=4, T=2048, C=1024, NH=16) on 8 trn2 NeuronCores.

Sharding: core = (head_group hg in {0,1}) x (batch b in {0..3}).
Each core computes qkv projection + attention + partial output projection for
its 8 heads of its batch; host sums the two head-group partials per batch and
adds the output bias.

Layout strategy (matmuls in float32r, ~TF32 accuracy, 4x fp32 speed):
  - qkv computed transposed: qkvT = W_slice @ x[b].T, so attention operands
    have head_size on partitions (contraction dim of QK^T).
  - The kernel is one interleaved loop: after the qkv projection of each
    512-token chunk n, the two 256-query attention tiles j = 2n, 2n+1 run
    (causality means they only need keys/values up to token 512(n+1)).  This
    overlaps the PE-bound projection with the ScalarE-bound softmax.
  - S^T = K @ Q^T per (head, key-tile, query-tile); 4 key-blocks are packed
    side-by-side in one [128, 1024] PSUM tile so a single ScalarE Exp covers
    them (amortizes ACT per-op overhead).  No max subtraction (scores are
    O(5), exp is safe); causal mask applied as one 0/1 multiply per
    (head, query-tile) on GpSimd over the two diagonal blocks.
  - qT is staged through a DRAM scratch and re-read per query tile (frees
    8KB/partition of SBUF, which the interleaving needs).
  - V is re-transposed to natural [t, hs] layout via PE transposes and
    augmented with a ones column, so the AV matmul also produces the softmax
    denominator for free: [V | 1]^T @ P^T = [O^T ; d].
  - 1/d is broadcast across partitions with a K=1 matmul against a ones
    vector, then applied with a vector multiply.
  - output projection contracts the core's 512 o-features; partial [T, C]
    result is summed on the host.
"""

import sys

sys.path.insert(0, "/opt/trn_rl_repo")

import numpy as np

import concourse.bacc as bacc
import concourse.bass as bass
import concourse.mybir as mybir
from concourse.bass_utils import run_bass_kernel_spmd
from concourse.masks import make_identity
from concourse.tile import TileContext

B, T, C, NH = 4, 2048, 1024, 16
HS = C // NH          # 64
HGF = 512             # features per head group (8 heads x 64)
QT = 256              # query tile (attention)
NJ = T // QT          # 8 query tiles
NKT = T // 128        # 16 key tiles
GRP = 4               # S-blocks packed per exp instruction
F32 = mybir.dt.float32
F32R = mybir.dt.float32r
Exp = mybir.ActivationFunctionType.Exp
Ident = mybir.ActivationFunctionType.Identity


def build_kernel():
    nc = bacc.Bacc(None, target_bir_lowering=False)
    xT = nc.dram_tensor("xT", (C, T), F32R, kind="ExternalInput")
    wqkvT = nc.dram_tensor("wqkvT", (C, 3 * HGF), F32R, kind="ExternalInput")
    bqkv = nc.dram_tensor("bqkv", (128, 12), F32, kind="ExternalInput")
    wprojT = nc.dram_tensor("wprojT", (HGF, C), F32R, kind="ExternalInput")
    mask01 = nc.dram_tensor("mask01", (128, 2 * QT), F32R, kind="ExternalInput")
    y = nc.dram_tensor("y", (T, C), F32, kind="ExternalOutput")

    with TileContext(nc) as tc:
        with (
            tc.tile_pool(name="outer", bufs=1) as outer,
            tc.tile_pool(name="work", bufs=1) as work,
            tc.tile_pool(name="dram", bufs=1, space="DRAM") as dram,
            tc.tile_pool(name="psum", bufs=1, space="PSUM") as psum,
        ):
            ident = outer.tile([128, 128], F32, name="ident")
            make_identity(nc, ident)
            ones_f = outer.tile([128, 2], F32, name="ones_f")
            nc.vector.memset(ones_f, 1.0)
            ones2 = outer.tile([128, 2], F32R, name="ones2")
            nc.vector.tensor_copy(ones2, ones_f)
            onesk_f = outer.tile([1, 64], F32, name="onesk_f")
            nc.vector.memset(onesk_f, 1.0)
            ones_k1 = outer.tile([1, 64], F32R, name="ones_k1")
            nc.vector.tensor_copy(ones_k1, onesk_f)

            # resident tensors
            k_t = [outer.tile([128, T], F32R, name=f"k{i}") for i in range(4)]
            # v_nat[pair][tk]: [tk 128, 130] = [headA 64 | 1 | headB 64 | 1]
            vnat = [
                [outer.tile([128, 130], F32R, name=f"v{p}_{i}") for i in range(NKT)]
                for p in range(4)
            ]
            wp_t = [outer.tile([128, C], F32R, name=f"wp{k}") for k in range(4)]
            for k in range(4):
                nc.gpsimd.dma_start(wp_t[k], wprojT[k * 128:(k + 1) * 128, :])
            # combined diagonal mask [128, 512]: left half = rows 0:128 of the
            # [256,256] triu mask, right half = rows 128:256
            mask_b = outer.tile([128, 2 * QT], F32R, name="mask_b")
            nc.gpsimd.dma_start(mask_b, mask01[:, :])

            q_dram = dram.tile([HGF, T], F32R, name="q_dram")

            # qkv weights + biases
            w_t = [work.tile([128, 3 * HGF], F32R, name=f"w{k}") for k in range(8)]
            wengs = [nc.gpsimd, nc.scalar, nc.sync]
            # free-dim chunks spread over engine queues, chunk-major so the
            # m=0..3 weight columns of every k arrive first (DMA cost scales
            # with free-dim bytes, not partitions)
            for ch in range(3):
                for k in range(8):
                    wengs[(8 * ch + k) % 3].dma_start(
                        w_t[k][:, ch * 512:(ch + 1) * 512],
                        wqkvT[k * 128:(k + 1) * 128, ch * 512:(ch + 1) * 512],
                    )
            bias_all = work.tile([128, 12], F32, name="bias_all")
            nc.sync.dma_start(bias_all, bqkv[:, :])
            bias_t = [bias_all[:, m:m + 1] for m in range(12)]

            for n in range(4):  # 512-token chunks
                # ---- qkv projection for chunk n ----
                x_n = []
                for k in range(8):
                    # one tag per k: all 8 k-tiles of a chunk are live at once
                    xt = work.tile([128, 512], F32R, tag=f"x{k}", bufs=1,
                                   name=f"x{n}_{k}")
                    xeng = nc.sync if k % 2 == 0 else nc.gpsimd
                    xeng.dma_start(
                        xt, xT[k * 128:(k + 1) * 128, n * 512:(n + 1) * 512]
                    )
                    x_n.append(xt)
                for m in range(12):
                    ps = psum.tile([128, 512], F32, tag="mix", bufs=2,
                                   name=f"ps{n}_{m}")
                    for k in range(8):
                        nc.tensor.matmul(
                            ps,
                            w_t[k][:, m * 128:(m + 1) * 128],
                            x_n[k],
                            start=(k == 0),
                            stop=(k == 7),
                        )
                    if m < 4:  # q -> DRAM staging
                        qs = work.tile([128, 512], F32R, tag="stage", bufs=3,
                                       name=f"qs{n}_{m}")
                        nc.scalar.activation(qs, ps, Ident, bias=bias_t[m])
                        nc.sync.dma_start(
                            q_dram[m * 128:(m + 1) * 128, n * 512:(n + 1) * 512], qs
                        )
                    elif m < 8:  # k resident
                        nc.scalar.activation(
                            k_t[m - 4][:, n * 512:(n + 1) * 512], ps, Ident,
                            bias=bias_t[m],
                        )
                    else:  # v -> transpose to natural layout
                        pair = m - 8
                        vtmp = work.tile([128, 512], F32, tag="vtmp", bufs=2,
                                         name=f"vt{n}_{m}")
                        nc.scalar.activation(vtmp, ps, Ident, bias=bias_t[m])
                        for t4 in range(4):
                            tk = 4 * n + t4
                            pst = psum.tile([128, 128], F32, tag="mix",
                                            bufs=2, name=f"pst{tk}_{m}")
                            nc.tensor.transpose(
                                pst, vtmp[:, t4 * 128:(t4 + 1) * 128], ident
                            )
                            vt = vnat[pair][tk]
                            # data cols {0..63, 65..128}; ones at {64, 129}
                            nc.vector.tensor_copy(
                                vt[:, 0:130].rearrange("p (g c) -> p g c", c=65)[:, :, 0:64],
                                pst.rearrange("p (g c) -> p g c", c=64),
                            )
                            nc.gpsimd.tensor_copy(
                                vt[:, 0:130].rearrange("p (g c) -> p g c", c=65)[:, :, 64:65],
                                ones2.rearrange("p (g c) -> p g c", c=1),
                            )

                # ---- attention for query tiles j = 2n, 2n+1 ----
                for j in (2 * n, 2 * n + 1):
                    q_j = []
                    for mq in range(4):
                        qt_ = work.tile([128, QT], F32R, tag=f"qj{mq}", bufs=2,
                                        name=f"qj{mq}_{j}")
                        nc.sync.dma_start(
                            qt_, q_dram[mq * 128:(mq + 1) * 128, j * QT:(j + 1) * QT]
                        )
                        q_j.append(qt_)
                    o_j = [
                        work.tile([128, QT], F32R, tag=f"o{i}", bufs=2,
                                  name=f"o{i}_{j}")
                        for i in range(4)
                    ]
                    for h in range(8):
                        pair, off = h // 2, 64 * (h % 2)
                        voff = 65 * (h % 2)
                        n_tk = 2 * (j + 1)
                        po = psum.tile([65, QT], F32, tag="po", bufs=2,
                                       name=f"po{j}_{h}")
                        for g in range((n_tk + GRP - 1) // GRP):
                            blk = min(GRP, n_tk - g * GRP)
                            sg = psum.tile([128, GRP * QT], F32, tag="big", bufs=2,
                                           name=f"sg{j}_{h}_{g}")
                            for bi in range(blk):
                                i = g * GRP + bi
                                nc.tensor.matmul(
                                    sg[:, bi * QT:(bi + 1) * QT],
                                    k_t[pair][off:off + 64, i * 128:(i + 1) * 128],
                                    q_j[h // 2][off:off + 64, :],
                                    start=True,
                                    stop=True,
                                )
                            pt = work.tile([128, GRP * QT], F32R, tag="pt", bufs=4,
                                           name=f"pt{j}_{h}_{g}")
                            nc.scalar.activation(
                                pt[:, :blk * QT], sg[:, :blk * QT], Exp, scale=0.125
                            )
                            if g * GRP <= 2 * j < (g + 1) * GRP:
                                # diagonal pair of blocks: one combined mask mul
                                pos = (2 * j - g * GRP) * QT
                                nc.gpsimd.tensor_mul(
                                    pt[:, pos:pos + 2 * QT],
                                    pt[:, pos:pos + 2 * QT],
                                    mask_b,
                                )
                            for bi in range(blk):
                                i = g * GRP + bi
                                nc.tensor.matmul(
                                    po,
                                    vnat[pair][i][:, voff:voff + 65],
                                    pt[:, bi * QT:(bi + 1) * QT],
                                    start=(i == 0),
                                    stop=(i == n_tk - 1),
                                )
                        dinv_r = work.tile([1, QT], F32R, tag="dinvr", bufs=2,
                                           name=f"dr{j}_{h}")
                        with nc.allow_low_precision(reason="fp32r matmul operand"):
                            nc.vector.reciprocal(dinv_r, po[64:65, :])
                        pb = psum.tile([64, QT], F32, tag="mix", bufs=2,
                                       name=f"pb{j}_{h}")
                        nc.tensor.matmul(pb, ones_k1, dinv_r, start=True, stop=True)
                        bc = work.tile([64, QT], F32, tag="bc", bufs=2,
                                       name=f"bc{j}_{h}")
                        nc.vector.tensor_copy(bc, pb)
                        nc.vector.tensor_mul(o_j[pair][off:off + 64, :], po[0:64, :], bc)
                    for mm in range(2):
                        for nn in range(2):
                            psy = psum.tile([128, 512], F32, tag="mix", bufs=2,
                                            name=f"py{j}_{mm}_{nn}")
                            for k4 in range(4):
                                nc.tensor.matmul(
                                    psy,
                                    o_j[k4][:, mm * 128:(mm + 1) * 128],
                                    wp_t[k4][:, nn * 512:(nn + 1) * 512],
                                    start=(k4 == 0),
                                    stop=(k4 == 3),
                                )
                            ysb = work.tile([128, 512], F32, tag="ysb", bufs=3,
                                            name=f"ys{j}_{mm}_{nn}")
                            nc.vector.tensor_copy(ysb, psy)
                            nc.sync.dma_start(
                                y[
                                    j * QT + mm * 128:j * QT + (mm + 1) * 128,
                                    nn * 512:(nn + 1) * 512,
                                ],
                                ysb,
                            )

    nc.finalize()
    return nc


_NC = None


def _get_nc():
    global _NC
    if _NC is None:
        _NC = build_kernel()
    return _NC


def kernel(x, Wqkv, bqkv, Wproj, bproj, _trace=False):
    x = np.asarray(x, dtype=np.float32)
    Wqkv = np.asarray(Wqkv, dtype=np.float32)
    bqkv = np.asarray(bqkv, dtype=np.float32)
    Wproj = np.asarray(Wproj, dtype=np.float32)
    bproj = np.asarray(bproj, dtype=np.float32)

    tri = np.triu(np.ones((2 * QT, 2 * QT), dtype=np.float32))[:, :QT]
    # combined diagonal mask: [rows 0:128 | rows 128:256] of the [256,256] triu
    mask = np.ascontiguousarray(np.concatenate([tri[0:128], tri[128:256]], axis=1))
    in_maps = []
    for hg in range(2):
        sl = slice(hg * HGF, (hg + 1) * HGF)
        rows = np.concatenate([Wqkv[sl], Wqkv[1024 + hg * HGF:1024 + (hg + 1) * HGF],
                               Wqkv[2048 + hg * HGF:2048 + (hg + 1) * HGF]])
        wqkvT = np.ascontiguousarray(rows.T)  # [C, 1536]
        bq = np.ascontiguousarray(np.concatenate(
            [bqkv[sl], bqkv[1024 + hg * HGF:1024 + (hg + 1) * HGF],
             bqkv[2048 + hg * HGF:2048 + (hg + 1) * HGF]]
        ).reshape(12, 128).T)
        wprojT = np.ascontiguousarray(Wproj[:, sl].T)  # [512, C]
        for b in range(B):
            in_maps.append(
                {
                    "xT": np.ascontiguousarray(x[b].T),
                    "wqkvT": wqkvT,
                    "bqkv": bq,
                    "wprojT": wprojT,
                    "mask01": mask,
                }
            )
    # core order: idx = hg * 4 + b
    res = run_bass_kernel_spmd(_get_nc(), in_maps, core_ids=list(range(8)),
                               trace=_trace)
    out = np.empty((B, T, C), dtype=np.float32)
    for b in range(B):
        out[b] = res.results[b]["y"] + res.results[4 + b]["y"] + bproj
    if _trace:
        return out, res
    return out


# revision 15
# speedup vs baseline: 16484.8680x; 16475.4346x over previous
"""Causal self-attention (B=4, T=2048, C=1024, NH=16) on 8 trn2 NeuronCores.

Sharding: core = (head_group hg in {0,1}) x (batch b in {0..3}).
Each core computes qkv projection + attention + partial output projection for
its 8 heads of its batch; host sums the two head-group partials per batch and
adds the output bias.

Layout strategy (matmuls in float32r, ~TF32 accuracy, 4x fp32 speed):
  - qkv computed transposed: qkvT = W_slice @ x[b].T, so attention operands
    have head_size on partitions (contraction dim of QK^T).
  - The kernel is one interleaved loop: after the qkv projection of each
    512-token chunk n, the two 256-query attention tiles j = 2n, 2n+1 run
    (causality means they only need keys/values up to token 512(n+1)).  This
    overlaps the PE-bound projection with the ScalarE-bound softmax.
  - S^T = K @ Q^T per (head, key-tile, query-tile); 4 key-blocks are packed
    side-by-side in one [128, 1024] PSUM tile so a single ScalarE Exp covers
    them (amortizes ACT per-op overhead).  No max subtraction (scores are
    O(5), exp is safe); causal mask applied as one 0/1 multiply per
    (head, query-tile) on GpSimd over the two diagonal blocks.
  - qT is staged through a DRAM scratch and re-read per query tile (frees
    8KB/partition of SBUF, which the interleaving needs).
  - V is re-transposed to natural [t, hs] layout via PE transposes and
    augmented with a ones column, so the AV matmul also produces the softmax
    denominator for free: [V | 1]^T @ P^T = [O^T ; d].
  - 1/d is broadcast across partitions with a K=1 matmul against a ones
    vector, then applied with a vector multiply.
  - output projection contracts the core's 512 o-features; partial [T, C]
    result is summed on the host.
"""

import sys

sys.path.insert(0, "/opt/trn_rl_repo")

import numpy as np

import concourse.bacc as bacc
import concourse.bass as bass
import concourse.mybir as mybir
from concourse.bass_utils import run_bass_kernel_spmd
from concourse.masks import make_identity
from concourse.tile import TileContext

B, T, C, NH = 4, 2048, 1024, 16
HS = C // NH          # 64
HGF = 512             # features per head group (8 heads x 64)
QT = 256              # query tile (attention)
NJ = T // QT          # 8 query tiles
NKT = T // 128        # 16 key tiles
GRP = 4               # S-blocks packed per exp instruction
F32 = mybir.dt.float32
F32R = mybir.dt.float32r
Exp = mybir.ActivationFunctionType.Exp
Ident = mybir.ActivationFunctionType.Identity


def build_kernel():
    nc = bacc.Bacc(None, target_bir_lowering=False)
    xT = nc.dram_tensor("xT", (C, T), F32R, kind="ExternalInput")
    wqkvT = nc.dram_tensor("wqkvT", (C, 3 * HGF), F32R, kind="ExternalInput")
    bqkv = nc.dram_tensor("bqkv", (128, 12), F32, kind="ExternalInput")
    wprojT = nc.dram_tensor("wprojT", (HGF, C), F32R, kind="ExternalInput")
    mask01 = nc.dram_tensor("mask01", (128, 2 * QT), F32R, kind="ExternalInput")
    y = nc.dram_tensor("y", (T, C), F32, kind="ExternalOutput")

    with TileContext(nc) as tc:
        with (
            tc.tile_pool(name="outer", bufs=1) as outer,
            tc.tile_pool(name="work", bufs=1) as work,
            tc.tile_pool(name="dram", bufs=1, space="DRAM") as dram,
            tc.tile_pool(name="psum", bufs=1, space="PSUM") as psum,
        ):
            ident = outer.tile([128, 128], F32, name="ident")
            make_identity(nc, ident)
            ones_f = outer.tile([128, 2], F32, name="ones_f")
            nc.vector.memset(ones_f, 1.0)
            ones2 = outer.tile([128, 2], F32R, name="ones2")
            nc.vector.tensor_copy(ones2, ones_f)
            onesk_f = outer.tile([1, 64], F32, name="onesk_f")
            nc.vector.memset(onesk_f, 1.0)
            ones_k1 = outer.tile([1, 64], F32R, name="ones_k1")
            nc.vector.tensor_copy(ones_k1, onesk_f)

            # resident tensors
            k_t = [outer.tile([128, T], F32R, name=f"k{i}") for i in range(4)]
            # v_nat[pair][tk]: [tk 128, 130] = [headA 64 | 1 | headB 64 | 1]
            vnat = [
                [outer.tile([128, 130], F32R, name=f"v{p}_{i}") for i in range(NKT)]
                for p in range(4)
            ]
            wp_t = [outer.tile([128, C], F32R, name=f"wp{k}") for k in range(4)]
            for k in range(4):
                nc.gpsimd.dma_start(wp_t[k], wprojT[k * 128:(k + 1) * 128, :])
            # combined diagonal mask [128, 512]: left half = rows 0:128 of the
            # [256,256] triu mask, right half = rows 128:256
            mask_b = outer.tile([128, 2 * QT], F32R, name="mask_b")
            nc.gpsimd.dma_start(mask_b, mask01[:, :])

            q_dram = dram.tile([HGF, T], F32R, name="q_dram")

            # qkv weights + biases
            w_t = [work.tile([128, 3 * HGF], F32R, name=f"w{k}") for k in range(8)]
            wengs = [nc.gpsimd, nc.scalar, nc.sync]
            # free-dim chunks spread over engine queues, chunk-major so the
            # m=0..3 weight columns of every k arrive first (DMA cost scales
            # with free-dim bytes, not partitions)
            for ch in range(3):
                for k in range(8):
                    wengs[(8 * ch + k) % 3].dma_start(
                        w_t[k][:, ch * 512:(ch + 1) * 512],
                        wqkvT[k * 128:(k + 1) * 128, ch * 512:(ch + 1) * 512],
                    )
            bias_all = work.tile([128, 12], F32, name="bias_all")
            nc.sync.dma_start(bias_all, bqkv[:, :])
            bias_t = [bias_all[:, m:m + 1] for m in range(12)]

            for n in range(4):  # 512-token chunks
                # ---- qkv projection for chunk n ----
                x_n = []
                for k in range(8):
                    # one tag per k: all 8 k-tiles of a chunk are live at once
                    xt = work.tile([128, 512], F32R, tag=f"x{k}", bufs=1,
                                   name=f"x{n}_{k}")
                    xeng = nc.sync if k % 2 == 0 else nc.gpsimd
                    xeng.dma_start(
                        xt, xT[k * 128:(k + 1) * 128, n * 512:(n + 1) * 512]
                    )
                    x_n.append(xt)
                for m in range(12):
                    ps = psum.tile([128, 512], F32, tag="o", bufs=4,
                                   name=f"ps{n}_{m}")
                    for k in range(8):
                        nc.tensor.matmul(
                            ps,
                            w_t[k][:, m * 128:(m + 1) * 128],
                            x_n[k],
                            start=(k == 0),
                            stop=(k == 7),
                        )
                    if m < 4:  # q -> DRAM staging
                        qs = work.tile([128, 512], F32R, tag="stage", bufs=3,
                                       name=f"qs{n}_{m}")
                        nc.scalar.activation(qs, ps, Ident, bias=bias_t[m])
                        nc.sync.dma_start(
                            q_dram[m * 128:(m + 1) * 128, n * 512:(n + 1) * 512], qs
                        )
                    elif m < 8:  # k resident
                        nc.scalar.activation(
                            k_t[m - 4][:, n * 512:(n + 1) * 512], ps, Ident,
                            bias=bias_t[m],
                        )
                    else:  # v -> transpose to natural layout
                        pair = m - 8
                        vtmp = work.tile([128, 512], F32, tag="vtmp", bufs=2,
                                         name=f"vt{n}_{m}")
                        nc.scalar.activation(vtmp, ps, Ident, bias=bias_t[m])
                        for t4 in range(4):
                            tk = 4 * n + t4
                            pst = psum.tile([128, 128], F32, tag="o",
                                            bufs=4, name=f"pst{tk}_{m}")
                            nc.tensor.transpose(
                                pst, vtmp[:, t4 * 128:(t4 + 1) * 128], ident
                            )
                            vt = vnat[pair][tk]
                            # data cols {0..63, 65..128}; ones at {64, 129}
                            nc.vector.tensor_copy(
                                vt[:, 0:130].rearrange("p (g c) -> p g c", c=65)[:, :, 0:64],
                                pst.rearrange("p (g c) -> p g c", c=64),
                            )
                            nc.gpsimd.tensor_copy(
                                vt[:, 0:130].rearrange("p (g c) -> p g c", c=65)[:, :, 64:65],
                                ones2.rearrange("p (g c) -> p g c", c=1),
                            )

                # ---- attention for query tiles j = 2n, 2n+1 ----
                for j in (2 * n, 2 * n + 1):
                    q_j = []
                    for mq in range(4):
                        qt_ = work.tile([128, QT], F32R, tag=f"qj{mq}", bufs=2,
                                        name=f"qj{mq}_{j}")
                        nc.sync.dma_start(
                            qt_, q_dram[mq * 128:(mq + 1) * 128, j * QT:(j + 1) * QT]
                        )
                        q_j.append(qt_)
                    o_j = [
                        work.tile([128, QT], F32R, tag=f"o{i}", bufs=2,
                                  name=f"o{i}_{j}")
                        for i in range(4)
                    ]
                    for h in range(8):
                        pair, off = h // 2, 64 * (h % 2)
                        voff = 65 * (h % 2)
                        n_tk = 2 * (j + 1)
                        po = psum.tile([65, QT], F32, tag="o", bufs=4,
                                       name=f"po{j}_{h}")
                        for g in range((n_tk + GRP - 1) // GRP):
                            blk = min(GRP, n_tk - g * GRP)
                            sg = psum.tile([128, GRP * QT], F32, tag="big", bufs=2,
                                           name=f"sg{j}_{h}_{g}")
                            for bi in range(blk):
                                i = g * GRP + bi
                                nc.tensor.matmul(
                                    sg[:, bi * QT:(bi + 1) * QT],
                                    k_t[pair][off:off + 64, i * 128:(i + 1) * 128],
                                    q_j[h // 2][off:off + 64, :],
                                    start=True,
                                    stop=True,
                                )
                            pt = work.tile([128, GRP * QT], F32R, tag="pt", bufs=3,
                                           name=f"pt{j}_{h}_{g}")
                            nc.scalar.activation(
                                pt[:, :blk * QT], sg[:, :blk * QT], Exp, scale=0.125
                            )
                            if g * GRP <= 2 * j < (g + 1) * GRP:
                                # diagonal pair of blocks: one combined mask mul
                                pos = (2 * j - g * GRP) * QT
                                nc.gpsimd.tensor_mul(
                                    pt[:, pos:pos + 2 * QT],
                                    pt[:, pos:pos + 2 * QT],
                                    mask_b,
                                )
                            for bi in range(blk):
                                i = g * GRP + bi
                                nc.tensor.matmul(
                                    po,
                                    vnat[pair][i][:, voff:voff + 65],
                                    pt[:, bi * QT:(bi + 1) * QT],
                                    start=(i == 0),
                                    stop=(i == n_tk - 1),
                                )
                        dinv_r = work.tile([1, QT], F32R, tag="dinvr", bufs=2,
                                           name=f"dr{j}_{h}")
                        with nc.allow_low_precision(reason="fp32r matmul operand"):
                            nc.vector.reciprocal(dinv_r, po[64:65, :])
                        pb = psum.tile([64, QT], F32, tag="o", bufs=4,
                                       name=f"pb{j}_{h}")
                        nc.tensor.matmul(pb, ones_k1, dinv_r, start=True, stop=True)
                        bc = work.tile([64, QT], F32, tag="bc", bufs=2,
                                       name=f"bc{j}_{h}")
                        nc.vector.tensor_copy(bc, pb)
                        nc.vector.tensor_mul(o_j[pair][off:off + 64, :], po[0:64, :], bc)
                    for mm in range(2):
                        for nn in range(2):
                            psy = psum.tile([128, 512], F32, tag="o", bufs=4,
                                            name=f"py{j}_{mm}_{nn}")
                            for k4 in range(4):
                                nc.tensor.matmul(
                                    psy,
                                    o_j[k4][:, mm * 128:(mm + 1) * 128],
                                    wp_t[k4][:, nn * 512:(nn + 1) * 512],
                                    start=(k4 == 0),
                                    stop=(k4 == 3),
                                )
                            ysb = work.tile([128, 512], F32, tag="ysb", bufs=3,
                                            name=f"ys{j}_{mm}_{nn}")
                            nc.vector.tensor_copy(ysb, psy)
                            nc.sync.dma_start(
                                y[
                                    j * QT + mm * 128:j * QT + (mm + 1) * 128,
                                    nn * 512:(nn + 1) * 512,
                                ],
                                ysb,
                            )

    nc.finalize()
    return nc


_NC = None


def _get_nc():
    global _NC
    if _NC is None:
        _NC = build_kernel()
    return _NC


def kernel(x, Wqkv, bqkv, Wproj, bproj, _trace=False):
    x = np.asarray(x, dtype=np.float32)
    Wqkv = np.asarray(Wqkv, dtype=np.float32)
    bqkv = np.asarray(bqkv, dtype=np.float32)
    Wproj = np.asarray(Wproj, dtype=np.float32)
    bproj = np.asarray(bproj, dtype=np.float32)

    tri = np.triu(np.ones((2 * QT, 2 * QT), dtype=np.float32))[:, :QT]
    # combined diagonal mask: [rows 0:128 | rows 128:256] of the [256,256] triu
    mask = np.ascontiguousarray(np.concatenate([tri[0:128], tri[128:256]], axis=1))
    in_maps = []
    for hg in range(2):
        sl = slice(hg * HGF, (hg + 1) * HGF)
        rows = np.concatenate([Wqkv[sl], Wqkv[1024 + hg * HGF:1024 + (hg + 1) * HGF],
                               Wqkv[2048 + hg * HGF:2048 + (hg + 1) * HGF]])
        wqkvT = np.ascontiguousarray(rows.T)  # [C, 1536]
        bq = np.ascontiguousarray(np.concatenate(
            [bqkv[sl], bqkv[1024 + hg * HGF:1024 + (hg + 1) * HGF],
             bqkv[2048 + hg * HGF:2048 + (hg + 1) * HGF]]
        ).reshape(12, 128).T)
        wprojT = np.ascontiguousarray(Wproj[:, sl].T)  # [512, C]
        for b in range(B):
            in_maps.append(
                {
                    "xT": np.ascontiguousarray(x[b].T),
                    "wqkvT": wqkvT,
                    "bqkv": bq,
                    "wprojT": wprojT,
                    "mask01": mask,
                }
            )
    # core order: idx = hg * 4 + b
    res = run_bass_kernel_spmd(_get_nc(), in_maps, core_ids=list(range(8)),
                               trace=_trace)
    out = np.empty((B, T, C), dtype=np.float32)
    for b in range(B):
        out[b] = res.results[b]["y"] + res.results[4 + b]["y"] + bproj
    if _trace:
        return out, res
    return out
